# revision 1
# baseline (speedup 1.0000x reference)
"""Trainium2 Bass kernel for nn_Block_56616258896419 (moe_routing).

Self-contained: takes FULL inputs (as from setup_inputs()), returns FULL
[4,1024,1024] f32 output. Internally shards across 8 NeuronCores:
  - tokens 8-way (core r: batch r//2, sequence half r%2) for attention/LN
  - experts 8-way (core r computes expert r over ALL tokens) for the MoE
Collectives: pairwise AllGather of K/V, 8-way AllGather of LN1'd
activations (transposed, bf16), 4x chunked 8-way ReduceScatter of the
prob-weighted expert outputs (overlapped with MoE compute).
"""
import numpy as np
import ml_dtypes

B, S, E, H, HD, NEXP, FF = 4, 1024, 1024, 16, 64, 8, 4096
NCORE = 8
TOK = 512          # tokens per core
TC = 256           # MoE token-chunk
NCHUNK = (B * S) // TC
EPS = 1e-5
BF16 = ml_dtypes.bfloat16

_CACHE = {}


def _build_program():
    import concourse.bacc as bacc
    import concourse.mybir as mybir
    import concourse.tile as tile

    dt = mybir.dt
    f32, bf = dt.float32, dt.bfloat16
    AF = mybir.ActivationFunctionType
    ALU = mybir.AluOpType

    nc = bacc.Bacc("TRN2", target_bir_lowering=False, debug=False,
                   num_devices=NCORE)

    # ---------------- I/O ----------------
    def inp(name, shape, d):
        return nc.dram_tensor(name, shape, d, kind="ExternalInput").ap()

    xT_d = inp("xT", [128, 2 * 4096], bf)         # x^T [own|partner], e-tiled
    xr_d = inp("xr", [128, 4 * 1024], f32)        # x token-major, tt blocks
    xr2_d = inp("xr2", [128, 4 * 1024], f32)      # x rows [b, 128r:128r+128]
    wqkv_d = inp("wqkv", [128, 8 * 3072], bf)     # [E,3E] e-tiled
    bqk_d = inp("bqk", [128, 16], f32)
    bv_d = inp("bv", [1, 1024], bf)
    wp_d = inp("wp", [128, 8 * 1024], bf)
    bp_d = inp("bp", [1, 1024], bf)
    masks_d = inp("masks", [4, 128, 256], bf)     # diag-pair 0/1 masks
    ident_d = inp("ident", [128, 128], bf)
    w1_d = inp("w1", [128, 8 * 4096], bf)         # ln1-folded, e-tiled
    b1_d = inp("b1", [128, 32], f32)
    w2_d = inp("w2", [128, 32 * 1024], bf)        # ff-tiled
    b2_d = inp("b2", [1, 1024], bf)
    wr_d = inp("wr", [128, 8 * 8], bf)            # ln1-folded, permuted
    br_d = inp("br", [1, 8], bf)
    g2_d = inp("g2", [128, 1024], f32)            # ln2_g replicated
    bl2_d = inp("bl2", [128, 1024], f32)          # ln2_b replicated
    out_d = nc.dram_tensor("out", [4, 128, 1024], f32,
                           kind="ExternalOutput").ap()

    # ---------------- internal DRAM ----------------
    hag_inA = nc.dram_tensor("hag_inA", [8, 128, 256], bf).ap()
    hag_inB = nc.dram_tensor("hag_inB", [8, 128, 256], bf).ap()
    hag_outA = nc.dram_tensor("hag_outA", [8, 8, 128, 256], bf,
                              addr_space="Shared").ap()
    hag_outB = nc.dram_tensor("hag_outB", [8, 8, 128, 256], bf,
                              addr_space="Shared").ap()
    rs_ins = [nc.dram_tensor(f"rs_in{g}", [1024, 1024], f32).ap()
              for g in range(4)]
    rs_outs = [nc.dram_tensor(f"rs_out{g}", [128, 1024], f32).ap()
               for g in range(4)]

    with tile.TileContext(nc) as tc:
        cpool_cm = tc.tile_pool(name="cpool", bufs=1, side="left")
        cpool = cpool_cm.__enter__()
        ones_row = cpool.tile([1, 128], bf)
        nc.vector.memset(ones_row[:], 1.0)
        ones_f = cpool.tile([1, 128], f32)
        nc.vector.memset(ones_f[:], 1.0)
        bqk_sb = cpool.tile([128, 16], f32)
        nc.sync.dma_start(bqk_sb[:], bqk_d[:])
        bv_sb = cpool.tile([1, 1024], bf)
        nc.sync.dma_start(bv_sb[:], bv_d[:])
        bp_sb = cpool.tile([1, 1024], bf)
        nc.sync.dma_start(bp_sb[:], bp_d[:])
        ident_sb = cpool.tile([128, 128], bf)
        nc.sync.dma_start(ident_sb[:], ident_d[:])
        wr_sb = cpool.tile([128, 64], bf)
        nc.sync.dma_start(wr_sb[:], wr_d[:])
        br_sb = cpool.tile([1, 8], bf)
        nc.sync.dma_start(br_sb[:], br_d[:])
        b1_sb = cpool.tile([128, 32], f32)
        nc.sync.dma_start(b1_sb[:], b1_d[:])
        b2_sb = cpool.tile([1, 1024], bf)
        nc.sync.dma_start(b2_sb[:], b2_d[:])

        # ===== phase 1: local K/V for BOTH interleave-halves, then Q =====
        qkv_cm = tc.tile_pool(name="qkv", bufs=1, side="right")
        qkv = qkv_cm.__enter__()
        xT_sb = qkv.tile([128, 8192], bf)
        nc.sync.dma_start(xT_sb[:], xT_d[:])
        wqkv_sb = qkv.tile([128, 24576], bf)
        for sect in (1, 2, 0):            # k first, then v, then q
            nc.sync.dma_start(
                wqkv_sb[:].rearrange("p (e s c) -> p e s c", e=8, s=3)[:, :, sect],
                wqkv_d[:].rearrange("p (e s c) -> p e s c", e=8, s=3)[:, :, sect])

        attn_cm = tc.tile_pool(name="attn", bufs=1, side="left")
        attn = attn_cm.__enter__()
        qT_sb = attn.tile([128, 4096], bf)
        kT_full = attn.tile([128, 8192], bf)     # [j][half*512 + s]
        v_full = attn.tile([128, 8192], bf)      # [u = half*4+tt][hd]

        with tc.tile_pool(name="ps_qkv", bufs=3, space="PSUM") as psq:
            for half in range(2):
                for j in range(8):
                    k_ps = psq.tile([128, 512], f32, tag="qk_ps")
                    for et in range(8):
                        nc.tensor.matmul(
                            k_ps[:],
                            wqkv_sb[:, et * 3072 + 1024 + j * 128:
                                    et * 3072 + 1024 + j * 128 + 128],
                            xT_sb[:, half * 4096 + et * 512:
                                  half * 4096 + et * 512 + 512],
                            start=(et == 0), stop=(et == 7))
                    nc.vector.tensor_scalar(
                        kT_full[:, j * 1024 + half * 512:
                                j * 1024 + half * 512 + 512], k_ps[:],
                        bqk_sb[:, 8 + j: 8 + j + 1], None, op0=ALU.add)
            for half in range(2):
                for tt in range(4):
                    for c in range(2):
                        v_ps = psq.tile([128, 512], f32, tag="v_ps")
                        for et in range(8):
                            nc.tensor.matmul(
                                v_ps[:],
                                xT_sb[:, half * 4096 + et * 512 + tt * 128:
                                      half * 4096 + et * 512 + tt * 128 + 128],
                                wqkv_sb[:, et * 3072 + 2048 + c * 512:
                                        et * 3072 + 2048 + c * 512 + 512],
                                start=(et == 0), stop=False)
                        nc.tensor.matmul(
                            v_ps[:], ones_row[:, 0:128],
                            bv_sb[:, c * 512: c * 512 + 512],
                            start=False, stop=True)
                        u = half * 4 + tt
                        nc.scalar.copy(
                            v_full[:, u * 1024 + c * 512:
                                   u * 1024 + c * 512 + 512], v_ps[:])
            # qT (own tokens = half 0)
            for j in range(8):
                q_ps = psq.tile([128, 512], f32, tag="qk_ps")
                for et in range(8):
                    nc.tensor.matmul(
                        q_ps[:],
                        wqkv_sb[:, et * 3072 + j * 128:
                                et * 3072 + j * 128 + 128],
                        xT_sb[:, et * 512: et * 512 + 512],
                        start=(et == 0), stop=(et == 7))
                nc.vector.tensor_scalar(
                    qT_sb[:, j * 512: j * 512 + 512], q_ps[:],
                    bqk_sb[:, j: j + 1], 0.125, op0=ALU.add, op1=ALU.mult)
        qkv_cm.__exit__(None, None, None)

        # ============ phase 2: attention ============
        # augmented V: per t-tile, 16 heads x (64 v-cols + 1 ones-col)
        v_aug = attn.tile([128, 8 * 1040], bf)
        for tt in range(8):
            nc.vector.tensor_copy(
                v_aug[:, tt * 1040: tt * 1040 + 1040]
                .rearrange("p (h d) -> p h d", d=65)[:, :, 0:64],
                v_full[:, tt * 1024: tt * 1024 + 1024]
                .rearrange("p (h d) -> p h d", d=64))
            nc.vector.memset(
                v_aug[:, tt * 1040: tt * 1040 + 1040]
                .rearrange("p (h d) -> p h d", d=65)[:, :, 64:65], 1.0)
        mask_sb = attn.tile([128, 1024], bf)
        for ss in range(4):
            nc.sync.dma_start(mask_sb[:, ss * 256: ss * 256 + 256],
                              masks_d[ss])

        proj_cm = tc.tile_pool(name="proj", bufs=1, side="right")
        projp = proj_cm.__enter__()
        catT_sb = projp.tile([128, 4096], bf)    # unnormalized heads^T

        with tc.tile_pool(name="sc", bufs=4, side="left") as scp, \
             tc.tile_pool(name="ps_sc", bufs=2, space="PSUM") as ps_sc, \
             tc.tile_pool(name="ps_av", bufs=3, space="PSUM") as ps_av:
            rc_tiles = []
            # block order within a subtile: non-diag (ph*ss+tl), diag at 2ss+ph
            def blk(ss, ph, tl):
                return 2 * ss + ph if tl == ss else ph * ss + tl
            for j in range(8):
                avps = [ps_av.tile([65, 512], f32, tag="av", name=f"av{j}_{k}")
                        for k in range(2)]
                first = [True, True]
                for ss in range(4):
                    scs = []
                    for h01 in range(2):
                        po = 64 * h01
                        sc_ps = ps_sc.tile([128, 1024], f32, tag="sc",
                                           name=f"sc{j}_{ss}_{h01}")
                        scs.append(sc_ps)
                        for ph in range(2):
                            for tl in range(ss + 1):
                                m = blk(ss, ph, tl)
                                nc.tensor.matmul(
                                    sc_ps[:, m * 128: m * 128 + 128],
                                    kT_full[po:po + 64,
                                            j * 1024 + ph * 512 + tl * 128:
                                            j * 1024 + ph * 512 + tl * 128 + 128],
                                    qT_sb[po:po + 64, j * 512 + ss * 128:
                                          j * 512 + ss * 128 + 128],
                                    start=True, stop=True)
                    exps = []
                    for h01 in range(2):
                        expT = scp.tile([128, 1024], bf, tag="expT",
                                        name=f"ex{j}_{ss}_{h01}")
                        exps.append(expT)
                        nc.scalar.activation(
                            expT[:, 0: (2 * ss + 2) * 128],
                            scs[h01][:, 0: (2 * ss + 2) * 128], AF.Exp)
                        nc.vector.tensor_tensor(
                            expT[:, 2 * ss * 128: 2 * ss * 128 + 256],
                            expT[:, 2 * ss * 128: 2 * ss * 128 + 256],
                            mask_sb[:, ss * 256: ss * 256 + 256], op=ALU.mult)
                    for h01 in range(2):
                        h = 2 * j + h01
                        for ph in range(2):
                            for tl in range(ss + 1):
                                m = blk(ss, ph, tl)
                                u = ph * 4 + tl
                                nc.tensor.matmul(
                                    avps[h01][:, ss * 128: ss * 128 + 128],
                                    v_aug[:, u * 1040 + h * 65:
                                          u * 1040 + h * 65 + 65],
                                    exps[h01][:, m * 128: m * 128 + 128],
                                    start=first[h01],
                                    stop=(ss == 3 and ph == 1 and tl == ss))
                                first[h01] = False
                rcs = []
                for h01 in range(2):
                    nc.scalar.copy(
                        catT_sb[64 * h01:64 * h01 + 64, j * 512: j * 512 + 512],
                        avps[h01][0:64, :])
                    sm_f = scp.tile([1, 512], f32, tag="sm_f", bufs=4,
                                    name=f"smf{j}_{h01}")
                    nc.scalar.copy(sm_f[:], avps[h01][64:65, :])
                    rc_f = scp.tile([1, 512], f32, tag="rc_f", bufs=4,
                                    name=f"rcf{j}_{h01}")
                    nc.vector.reciprocal_approx_fast(rc_f[:], sm_f[:])
                    rc_h = scp.tile([1, 512], bf, tag="rc_h", bufs=4,
                                    name=f"rc{j}_{h01}")
                    nc.vector.tensor_copy(rc_h[:], rc_f[:])
                    rcs.append(rc_h)
                # PE warm fillers (keep HAM at full clock through ACT-bound loop)
                for wi in range(6):
                    wm_ps = ps_av.tile([65, 512], f32, tag="wmv",
                                       name=f"wv{j}_{wi}", bufs=1)
                    nc.tensor.matmul(wm_ps[0:64, :], ident_sb[:, 0:64],
                                     kT_full[:, 0:512], start=True, stop=True)
                # normalize this head-pair's catT block (K=1 bcast matmuls)
                bc_ps = ps_sc.tile([128, 1024], f32, tag="sc",
                                   name=f"bc{j}")[:, 0:512]
                nc.tensor.matmul(bc_ps[0:64, :], ones_row[:, 0:64],
                                 rcs[0][:], start=True, stop=True)
                nc.tensor.matmul(bc_ps[64:128, :], ones_row[:, 0:64],
                                 rcs[1][:], start=True, stop=True)
                nc.vector.tensor_tensor(
                    catT_sb[:, j * 512: j * 512 + 512],
                    catT_sb[:, j * 512: j * 512 + 512], bc_ps[:],
                    op=ALU.mult)
        attn_cm.__exit__(None, None, None)

        # MoE weights prefetch - overlaps proj + LN1 + h-AllGather
        moe_cm = tc.tile_pool(name="moe", bufs=1, side="left")
        moe = moe_cm.__enter__()
        w1_sb = moe.tile([128, 32768], bf)
        w2_sb = moe.tile([128, 32768], bf)
        for et in range(8):
            nc.sync.dma_start(w1_sb[:, et * 4096: et * 4096 + 4096],
                              w1_d[:, et * 4096: et * 4096 + 4096])
        for ft8 in range(8):
            nc.sync.dma_start(w2_sb[:, ft8 * 4096: ft8 * 4096 + 4096],
                              w2_d[:, ft8 * 4096: ft8 * 4096 + 4096])

        wp_sb = projp.tile([128, 8192], bf)
        nc.sync.dma_start(wp_sb[:], wp_d[:])
        x_sb = projp.tile([128, 4096], f32)
        nc.sync.dma_start(x_sb[:], xr_d[:])
        h_sb = projp.tile([128, 4096], bf)
        hT_stage = projp.tile([128, 4096], bf)

        with tc.tile_pool(name="prw", bufs=2, side="left") as prp, \
             tc.tile_pool(name="ps_pr", bufs=4, space="PSUM") as ps_pr:
            for tt in range(4):
                y_sb = prp.tile([128, 1024], f32, tag="y")
                for ec in range(2):
                    ao_ps = ps_pr.tile([128, 512], f32, tag="ao")
                    for jc in range(8):
                        nc.tensor.matmul(
                            ao_ps[:],
                            catT_sb[:, jc * 512 + tt * 128:
                                    jc * 512 + tt * 128 + 128],
                            wp_sb[:, jc * 1024 + ec * 512:
                                  jc * 1024 + ec * 512 + 512],
                            start=(jc == 0), stop=False)
                    nc.tensor.matmul(
                        ao_ps[:], ones_row[:, 0:128],
                        bp_sb[:, ec * 512: ec * 512 + 512],
                        start=False, stop=True)
                    nc.vector.tensor_tensor(
                        y_sb[:, ec * 512: ec * 512 + 512], ao_ps[:],
                        x_sb[:, tt * 1024 + ec * 512: tt * 1024 + ec * 512 + 512],
                        op=ALU.add)
                # LN1 stats
                mean = prp.tile([128, 1], f32, tag="mean")
                nc.vector.reduce_sum(mean[:], y_sb[:], axis=mybir.AxisListType.X)
                nc.vector.tensor_scalar_mul(mean[:], mean[:], 1.0 / 1024.0)
                sq = prp.tile([128, 1024], bf, tag="sq")
                sqs = prp.tile([128, 1], f32, tag="sqs")
                nc.scalar.activation(sq[:], y_sb[:], AF.Square,
                                     accum_out=sqs[:])
                m2 = prp.tile([128, 1], f32, tag="m2")
                nc.scalar.activation(m2[:], mean[:], AF.Square)
                var = prp.tile([128, 1], f32, tag="var")
                nc.vector.tensor_scalar(var[:], sqs[:], 1.0 / 1024.0, EPS,
                                        op0=ALU.mult, op1=ALU.add)
                nc.vector.tensor_tensor(var[:], var[:], m2[:], op=ALU.subtract)
                std = prp.tile([128, 1], f32, tag="std")
                nc.scalar.activation(std[:], var[:], AF.Sqrt)
                rstd = prp.tile([128, 1], f32, tag="rstd")
                nc.vector.reciprocal(rstd[:], std[:])
                nc.vector.tensor_scalar(
                    h_sb[:, tt * 1024: tt * 1024 + 1024], y_sb[:],
                    mean[:], rstd[:], op0=ALU.subtract, op1=ALU.mult)
                # transpose h tile -> hT
                for et in range(8):
                    tp = ps_pr.tile([128, 128], bf, tag="tp")
                    nc.tensor.transpose(
                        tp[:], h_sb[:, tt * 1024 + et * 128:
                                    tt * 1024 + et * 128 + 128], ident_sb[:])
                    nc.scalar.copy(
                        hT_stage[:, et * 512 + tt * 128:
                                 et * 512 + tt * 128 + 128], tp[:])
                if tt == 1:
                    for et in range(8):
                        nc.gpsimd.dma_start(
                            hag_inA[et],
                            hT_stage[:, et * 512: et * 512 + 256])
                    nc.gpsimd.collective_compute(
                        "AllGather", mybir.AluOpType.bypass,
                        replica_groups=[list(range(8))],
                        ins=[hag_inA.opt()], outs=[hag_outA.opt()])
                if tt == 3:
                    for et in range(8):
                        nc.gpsimd.dma_start(
                            hag_inB[et],
                            hT_stage[:, et * 512 + 256: et * 512 + 512])
                    nc.gpsimd.collective_compute(
                        "AllGather", mybir.AluOpType.bypass,
                        replica_groups=[list(range(8))],
                        ins=[hag_inB.opt()], outs=[hag_outB.opt()])
            # (b) PE warm-keepers over the h-AG wait
            for wi in range(16):
                wm_ps = ps_pr.tile([128, 512], f32, tag="ao", name=f"wm{wi}")
                nc.tensor.matmul(wm_ps[:], catT_sb[:, 0:128],
                                 wp_sb[:, 0:512], start=True, stop=True)
        proj_cm.__exit__(None, None, None)

        # ============ phase 3: MoE (expert r over all tokens) ============
        with tc.tile_pool(name="mchunk", bufs=2, side="left") as mck, \
             tc.tile_pool(name="ps_md", bufs=2, space="PSUM") as ps_md, \
             tc.tile_pool(name="ps_eo", bufs=2, space="PSUM") as ps_eo:
            order = []
            for qq in range(4):
                order += [4 * qq, 4 * qq + 2, 4 * qq + 1, 4 * qq + 3]
            gcount = {0: 0, 1: 0, 2: 0, 3: 0}
            for ci in order:
                hT_c = mck.tile([128, 2048], bf, tag="hT_c")
                hsrc = hag_outA if ci % 2 == 0 else hag_outB
                for et in range(8):
                    nc.gpsimd.dma_start(
                        hT_c[:, et * 256: et * 256 + 256],
                        hsrc[ci // 2, et])
                pcol = mck.tile([128, 2], f32, tag="pcol")
                for th in range(2):
                    lg_ps = ps_eo.tile([128, 8], f32, tag="lg")
                    for et in range(8):
                        nc.tensor.matmul(
                            lg_ps[:],
                            hT_c[:, et * 256 + th * 128: et * 256 + th * 128 + 128],
                            wr_sb[:, et * 8: et * 8 + 8],
                            start=(et == 0), stop=False)
                    nc.tensor.matmul(lg_ps[:], ones_row[:, 0:128], br_sb[:],
                                     start=False, stop=True)
                    pe = mck.tile([128, 8], f32, tag="pe")
                    ps = mck.tile([128, 1], f32, tag="ps")
                    nc.scalar.activation(pe[:], lg_ps[:], AF.Exp,
                                         accum_out=ps[:])
                    pr = mck.tile([128, 1], f32, tag="pr")
                    nc.vector.reciprocal(pr[:], ps[:])
                    nc.vector.tensor_tensor(pcol[:, th:th + 1], pe[:, 0:1],
                                            pr[:], op=ALU.mult)
                midT = mck.tile([128, 8192], bf, tag="midT", bufs=2)
                for ft in range(32):
                    md_ps = ps_md.tile([128, 256], f32, tag="md")
                    for et in range(8):
                        nc.tensor.matmul(
                            md_ps[:],
                            w1_sb[:, et * 4096 + ft * 128: et * 4096 + ft * 128 + 128],
                            hT_c[:, et * 256: et * 256 + 256],
                            start=(et == 0), stop=(et == 7))
                    if ft % 2 == 0:
                        nc.scalar.activation(
                            midT[:, ft * 256: ft * 256 + 256], md_ps[:],
                            AF.Relu, bias=b1_sb[:, ft: ft + 1])
                    else:
                        nc.vector.tensor_scalar(
                            midT[:, ft * 256: ft * 256 + 256], md_ps[:],
                            b1_sb[:, ft: ft + 1], 0.0,
                            op0=ALU.add, op1=ALU.max)
                eo_sb = mck.tile([128, 2048], f32, tag="eo", bufs=2)
                for th in range(2):
                    for ec in range(2):
                        eo_ps = ps_eo.tile([128, 512], f32, tag="eo_ps")
                        for ft in range(32):
                            nc.tensor.matmul(
                                eo_ps[:],
                                midT[:, ft * 256 + th * 128: ft * 256 + th * 128 + 128],
                                w2_sb[:, ft * 1024 + ec * 512: ft * 1024 + ec * 512 + 512],
                                start=(ft == 0), stop=False)
                        nc.tensor.matmul(
                            eo_ps[:], ones_row[:, 0:128],
                            b2_sb[:, ec * 512: ec * 512 + 512],
                            start=False, stop=True)
                        if ec == 0:
                            nc.scalar.activation(
                                eo_sb[:, th * 1024 + ec * 512:
                                      th * 1024 + ec * 512 + 512],
                                eo_ps[:], AF.Identity,
                                scale=pcol[:, th: th + 1])
                        else:
                            nc.vector.tensor_scalar_mul(
                                eo_sb[:, th * 1024 + ec * 512:
                                      th * 1024 + ec * 512 + 512],
                                eo_ps[:], pcol[:, th: th + 1])
                g, gi = ci // 4, ci % 4
                for th in range(2):
                    nc.sync.dma_start(
                        rs_ins[g][gi * 256 + th * 128: gi * 256 + th * 128 + 128, :],
                        eo_sb[:, th * 1024: th * 1024 + 1024])
                gcount[g] += 1
                if gcount[g] == 4:
                    nc.gpsimd.collective_compute(
                        "ReduceScatter", mybir.AluOpType.add,
                        replica_groups=[list(range(8))],
                        ins=[rs_ins[g].opt()], outs=[rs_outs[g].opt()])
        moe_cm.__exit__(None, None, None)

        # ============ phase 4: residual + LN2 (per RS group/batch) ============
        with tc.tile_pool(name="fin", bufs=2, side="left") as fin:
            x2_sb = fin.tile([128, 4096], f32, bufs=1)
            nc.sync.dma_start(x2_sb[:], xr2_d[:])
            g2_sb = fin.tile([128, 1024], f32, bufs=1)
            nc.sync.dma_start(g2_sb[:], g2_d[:])
            bl2_sb = fin.tile([128, 1024], f32, bufs=1)
            nc.sync.dma_start(bl2_sb[:], bl2_d[:])
            for g in range(4):
                y2 = fin.tile([128, 1024], f32, tag="y2")
                nc.sync.dma_start(y2[:], rs_outs[g][:])
                nc.vector.tensor_tensor(
                    y2[:], y2[:], x2_sb[:, g * 1024: g * 1024 + 1024],
                    op=ALU.add)
                mean = fin.tile([128, 1], f32, tag="mean2")
                nc.vector.reduce_sum(mean[:], y2[:], axis=mybir.AxisListType.X)
                nc.vector.tensor_scalar_mul(mean[:], mean[:], 1.0 / 1024.0)
                sq = fin.tile([128, 1024], f32, tag="sq2")
                sqs = fin.tile([128, 1], f32, tag="sqs2")
                nc.scalar.activation(sq[:], y2[:], AF.Square, accum_out=sqs[:])
                m2 = fin.tile([128, 1], f32, tag="m22")
                nc.scalar.activation(m2[:], mean[:], AF.Square)
                var = fin.tile([128, 1], f32, tag="var2")
                nc.vector.tensor_scalar(var[:], sqs[:], 1.0 / 1024.0, EPS,
                                        op0=ALU.mult, op1=ALU.add)
                nc.vector.tensor_tensor(var[:], var[:], m2[:], op=ALU.subtract)
                std = fin.tile([128, 1], f32, tag="std2")
                nc.scalar.activation(std[:], var[:], AF.Sqrt)
                rstd = fin.tile([128, 1], f32, tag="rstd2")
                nc.vector.reciprocal(rstd[:], std[:])
                on = fin.tile([128, 1024], f32, tag="on")
                nc.vector.tensor_scalar(on[:], y2[:], mean[:], rstd[:],
                                        op0=ALU.subtract, op1=ALU.mult)
                nc.vector.tensor_tensor(on[:], on[:], g2_sb[:], op=ALU.mult)
                nc.vector.tensor_tensor(on[:], on[:], bl2_sb[:], op=ALU.add)
                nc.sync.dma_start(out_d[g], on[:])
        cpool_cm.__exit__(None, None, None)
    nc.compile()
    return nc


def _prep_inputs(inputs):
    f = np.float32
    x = np.asarray(inputs["x"], f)
    wq, bq = np.asarray(inputs["wq"], f), np.asarray(inputs["bq"], f)
    wk, bk = np.asarray(inputs["wk"], f), np.asarray(inputs["bk"], f)
    wv, bv = np.asarray(inputs["wv"], f), np.asarray(inputs["bv"], f)
    wp, bp = np.asarray(inputs["wp"], f), np.asarray(inputs["bp"], f)
    ln1_g, ln1_b = np.asarray(inputs["ln1_g"], f), np.asarray(inputs["ln1_b"], f)
    ln2_g, ln2_b = np.asarray(inputs["ln2_g"], f), np.asarray(inputs["ln2_b"], f)
    wr, br = np.asarray(inputs["wr"], f), np.asarray(inputs["br"], f)
    w1, b1 = np.asarray(inputs["w1"], f), np.asarray(inputs["b1"], f)
    w2, b2 = np.asarray(inputs["w2"], f), np.asarray(inputs["b2"], f)

    def etile(a):  # [E, M] -> [128, 8*M]
        M = a.shape[1]
        return np.ascontiguousarray(
            a.reshape(8, 128, M).transpose(1, 0, 2).reshape(128, 8 * M))

    wq_f = wq.transpose(1, 0, 2).reshape(E, E)   # [e, h*64+d]
    wk_f = wk.transpose(1, 0, 2).reshape(E, E)
    wv_f = wv.transpose(1, 0, 2).reshape(E, E)
    wqkv = np.concatenate([wq_f, wk_f, wv_f], axis=1)        # [E, 3E]
    wqkv_t = etile(wqkv).astype(BF16)                        # [128, 8*3072]
    bqk = np.concatenate([bq.reshape(-1).reshape(8, 128).T,
                          bk.reshape(-1).reshape(8, 128).T], axis=1).astype(f)
    wp_t = etile(wp).astype(BF16)                            # [128, 8*1024]
    w1e = (ln1_g[:, None] * w1).astype(f)                    # [n,E,FF]
    b1e = b1 + ln1_b @ w1                                    # [n,FF]
    wre = (ln1_g[:, None] * wr).astype(f)                    # [E,8]
    bre = br + ln1_b @ wr                                    # [8]
    ident = np.eye(128, dtype=BF16)

    in_maps = []
    for r in range(NCORE):
        b, p = r // 2, r % 2
        # interleaved token assignment: local s_loc <-> orig row 2*s_loc + p
        xs = np.ascontiguousarray(x[b, p::2, :])             # [512, E]
        xpart = np.ascontiguousarray(x[b, 1 - p::2, :])      # partner tokens
        xT_t = np.concatenate(
            [etile(np.ascontiguousarray(xs.T)),
             etile(np.ascontiguousarray(xpart.T))], axis=1).astype(BF16)
        xr_t = np.ascontiguousarray(
            xs.reshape(4, 128, 1024).transpose(1, 0, 2).reshape(128, 4096), f)
        # final-phase x rows: vt rows [128r,128r+128) of every batch
        sv = 128 * r + np.arange(128)
        orig_s = 2 * (sv % 512) + sv // 512
        xr2_t = np.ascontiguousarray(
            x[:, orig_s, :].transpose(1, 0, 2).reshape(128, 4096), f)
        # diagonal causal masks: half0 = own parity keys, half1 = partner
        masks = np.zeros((4, 128, 256), BF16)
        ti = np.arange(128)
        sj = np.arange(128)
        own = (ti[:, None] <= sj[None, :])
        part = (ti[:, None] <= sj[None, :]) if p == 1 else                (ti[:, None] < sj[None, :])
        for ss in range(4):
            masks[ss][:, 0:128] = own.astype(BF16)
            masks[ss][:, 128:256] = part.astype(BF16)
        perm = [r] + [i for i in range(NEXP) if i != r]
        wr_p = etile(wre[:, perm]).astype(BF16)              # [128, 8*8]
        br_p = bre[perm].reshape(1, 8).astype(BF16)
        w1_t = etile(w1e[r]).astype(BF16)                    # [128, 8*4096]
        b1_t = np.ascontiguousarray(
            b1e[r].reshape(32, 128).T, f)                    # [128, 32]
        w2_t = np.ascontiguousarray(
            w2[r].reshape(32, 128, 1024).transpose(1, 0, 2)
            .reshape(128, 32 * 1024)).astype(BF16)
        in_maps.append({
            "xT": xT_t, "xr": xr_t, "xr2": xr2_t, "wqkv": wqkv_t, "bqk": bqk,
            "bv": bv.reshape(1, E).astype(BF16),
            "wp": wp_t, "bp": bp.reshape(1, E).astype(BF16),
            "masks": masks, "ident": ident,
            "w1": w1_t, "b1": b1_t, "w2": w2_t,
            "b2": b2[r].reshape(1, E).astype(BF16),
            "wr": wr_p, "br": br_p,
            "g2": np.broadcast_to(ln2_g, (128, E)).astype(f).copy(),
            "bl2": np.broadcast_to(ln2_b, (128, E)).astype(f).copy(),
        })
    return in_maps


def kernel(**inputs):
    from concourse import bass_utils
    if "nc" not in _CACHE:
        _CACHE["nc"] = _build_program()
    nc = _CACHE["nc"]
    in_maps = _prep_inputs(inputs)
    res = bass_utils.run_bass_kernel_spmd(
        nc, in_maps, core_ids=list(range(NCORE)))
    # core r returns vt rows [128r, 128r+128) of every batch (interleaved map)
    full = np.empty((B, S, E), np.float32)
    for r in range(NCORE):
        o = res.results[r]["out"]                            # [4, 128, 1024]
        sv = 128 * r + np.arange(128)
        orig_s = 2 * (sv % 512) + sv // 512
        full[:, orig_s, :] = o
    return full



# revision 7
# speedup vs baseline: 1.4866x; 1.4866x over previous
"""Trainium2 Bass kernel for nn_Block_56616258896419 (moe_routing).

Self-contained: takes FULL inputs (as from setup_inputs()), returns FULL
[4,1024,1024] f32 output. Internally shards across 8 NeuronCores:
  - tokens 8-way (core r: batch r//2, sequence half r%2) for attention/LN
  - experts 8-way (core r computes expert r over ALL tokens) for the MoE
Collectives: pairwise AllGather of K/V, 8-way AllGather of LN1'd
activations (transposed, bf16), 4x chunked 8-way ReduceScatter of the
prob-weighted expert outputs (overlapped with MoE compute).
"""
import numpy as np
import ml_dtypes

B, S, E, H, HD, NEXP, FF = 4, 1024, 1024, 16, 64, 8, 4096
NCORE = 8
TOK = 512          # tokens per core
TC = 256           # MoE token-chunk
NCHUNK = (B * S) // TC
EPS = 1e-5
BF16 = ml_dtypes.bfloat16
FP8 = ml_dtypes.float8_e4m3
WSCALE = 16.0          # fp8 prescale for w1/w2 (keeps weights in normal range)

_CACHE = {}


def _build_program():
    import concourse.bacc as bacc
    import concourse.mybir as mybir
    import concourse.tile as tile

    dt = mybir.dt
    f32, bf, f8 = dt.float32, dt.bfloat16, dt.float8e4
    AF = mybir.ActivationFunctionType
    ALU = mybir.AluOpType
    DR = mybir.MatmulPerfMode.DoubleRow

    nc = bacc.Bacc("TRN2", target_bir_lowering=False, debug=False,
                   num_devices=NCORE)

    # ---------------- I/O ----------------
    def inp(name, shape, d):
        return nc.dram_tensor(name, shape, d, kind="ExternalInput").ap()

    xT_d = inp("xT", [128, 2 * 4096], bf)         # x^T [own|partner], e-tiled
    xr_d = inp("xr", [128, 4 * 1024], f32)        # x token-major, tt blocks
    xr2_d = inp("xr2", [128, 4 * 1024], f32)      # x rows [b, 128r:128r+128]
    wqkv_d = inp("wqkv", [128, 8 * 3072], bf)     # [E,3E] e-tiled
    bqk_d = inp("bqk", [128, 16], f32)
    bv_d = inp("bv", [1, 1024], bf)
    wp_d = inp("wp", [128, 8 * 1024], bf)
    bp_d = inp("bp", [1, 1024], bf)
    masks_d = inp("masks", [4, 128, 256], bf)     # diag-pair 0/1 masks
    ident_d = inp("ident", [128, 128], bf)
    w1_d = inp("w1", [128, 8 * 4096], f8)         # ln1-folded, e-tiled, x16
    b1_d = inp("b1", [128, 32], f32)
    w2_d = inp("w2", [128, 32 * 1024], f8)        # ff-tiled, x16
    b2_d = inp("b2", [1, 1024], bf)
    wr_d = inp("wr", [128, 8 * 8], bf)            # ln1-folded, permuted
    br_d = inp("br", [1, 8], bf)
    g2_d = inp("g2", [128, 1024], f32)            # ln2_g replicated
    bl2_d = inp("bl2", [128, 1024], f32)          # ln2_b replicated
    out_d = nc.dram_tensor("out", [4, 128, 1024], f32,
                           kind="ExternalOutput").ap()

    # ---------------- internal DRAM ----------------
    hag_inA = nc.dram_tensor("hag_inA", [8, 128, 256], bf).ap()
    hag_inB = nc.dram_tensor("hag_inB", [8, 128, 256], bf).ap()
    hag_outA = nc.dram_tensor("hag_outA", [8, 8, 128, 256], bf,
                              addr_space="Shared").ap()
    hag_outB = nc.dram_tensor("hag_outB", [8, 8, 128, 256], bf,
                              addr_space="Shared").ap()
    rs_ins = [nc.dram_tensor(f"rs_in{g}", [1024, 1024], f32).ap()
              for g in range(4)]
    rs_outs = [nc.dram_tensor(f"rs_out{g}", [128, 1024], f32).ap()
               for g in range(4)]

    with tile.TileContext(nc) as tc:
        cpool_cm = tc.tile_pool(name="cpool", bufs=1, side="left")
        cpool = cpool_cm.__enter__()
        ones_row = cpool.tile([1, 128], bf)
        nc.vector.memset(ones_row[:], 1.0)
        ones_f = cpool.tile([1, 128], f32)
        nc.vector.memset(ones_f[:], 1.0)
        bqk_sb = cpool.tile([128, 16], f32)
        nc.sync.dma_start(bqk_sb[:], bqk_d[:])
        bv_sb = cpool.tile([1, 1024], bf)
        nc.sync.dma_start(bv_sb[:], bv_d[:])
        bp_sb = cpool.tile([1, 1024], bf)
        nc.sync.dma_start(bp_sb[:], bp_d[:])
        ident_sb = cpool.tile([128, 128], bf)
        nc.sync.dma_start(ident_sb[:], ident_d[:])
        wr_sb = cpool.tile([128, 64], bf)
        nc.sync.dma_start(wr_sb[:], wr_d[:])
        br_sb = cpool.tile([1, 8], bf)
        nc.sync.dma_start(br_sb[:], br_d[:])
        b1_sb = cpool.tile([128, 32], f32)
        nc.sync.dma_start(b1_sb[:], b1_d[:])
        b2_sb = cpool.tile([1, 1024], bf)
        nc.sync.dma_start(b2_sb[:], b2_d[:])

        # ===== phase 1: local K/V for BOTH interleave-halves, then Q =====
        qkv_cm = tc.tile_pool(name="qkv", bufs=1, side="right")
        qkv = qkv_cm.__enter__()
        xT_sb = qkv.tile([128, 8192], bf)
        nc.sync.dma_start(xT_sb[:], xT_d[:])
        wqkv_sb = qkv.tile([128, 24576], bf)
        for sect in (1, 2, 0):            # k first, then v, then q
            nc.sync.dma_start(
                wqkv_sb[:].rearrange("p (e s c) -> p e s c", e=8, s=3)[:, :, sect],
                wqkv_d[:].rearrange("p (e s c) -> p e s c", e=8, s=3)[:, :, sect])

        attn_cm = tc.tile_pool(name="attn", bufs=1, side="left")
        attn = attn_cm.__enter__()
        qT_sb = attn.tile([128, 4096], bf)
        kT_full = attn.tile([128, 8192], bf)     # [j][half*512 + s]
        v_full = attn.tile([128, 8192], bf)      # [u = half*4+tt][hd]

        with tc.tile_pool(name="ps_qkv", bufs=3, space="PSUM") as psq:
            for half in range(2):
                for j in range(8):
                    k_ps = psq.tile([128, 512], f32, tag="qk_ps")
                    for et in range(8):
                        nc.tensor.matmul(
                            k_ps[:],
                            wqkv_sb[:, et * 3072 + 1024 + j * 128:
                                    et * 3072 + 1024 + j * 128 + 128],
                            xT_sb[:, half * 4096 + et * 512:
                                  half * 4096 + et * 512 + 512],
                            start=(et == 0), stop=(et == 7))
                    nc.vector.tensor_scalar(
                        kT_full[:, j * 1024 + half * 512:
                                j * 1024 + half * 512 + 512], k_ps[:],
                        bqk_sb[:, 8 + j: 8 + j + 1], None, op0=ALU.add)
            for half in range(2):
                for tt in range(4):
                    for c in range(2):
                        v_ps = psq.tile([128, 512], f32, tag="v_ps")
                        for et in range(8):
                            nc.tensor.matmul(
                                v_ps[:],
                                xT_sb[:, half * 4096 + et * 512 + tt * 128:
                                      half * 4096 + et * 512 + tt * 128 + 128],
                                wqkv_sb[:, et * 3072 + 2048 + c * 512:
                                        et * 3072 + 2048 + c * 512 + 512],
                                start=(et == 0), stop=False)
                        nc.tensor.matmul(
                            v_ps[:], ones_row[:, 0:128],
                            bv_sb[:, c * 512: c * 512 + 512],
                            start=False, stop=True)
                        u = half * 4 + tt
                        nc.scalar.copy(
                            v_full[:, u * 1024 + c * 512:
                                   u * 1024 + c * 512 + 512], v_ps[:])
            # qT (own tokens = half 0)
            for j in range(8):
                q_ps = psq.tile([128, 512], f32, tag="qk_ps")
                for et in range(8):
                    nc.tensor.matmul(
                        q_ps[:],
                        wqkv_sb[:, et * 3072 + j * 128:
                                et * 3072 + j * 128 + 128],
                        xT_sb[:, et * 512: et * 512 + 512],
                        start=(et == 0), stop=(et == 7))
                nc.vector.tensor_scalar(
                    qT_sb[:, j * 512: j * 512 + 512], q_ps[:],
                    bqk_sb[:, j: j + 1], 0.125, op0=ALU.add, op1=ALU.mult)
        qkv_cm.__exit__(None, None, None)

        # ============ phase 2: attention ============
        # augmented V: per t-tile, 16 heads x (64 v-cols + 1 ones-col)
        v_aug = attn.tile([128, 8 * 1040], bf)
        for tt in range(8):
            nc.vector.tensor_copy(
                v_aug[:, tt * 1040: tt * 1040 + 1040]
                .rearrange("p (h d) -> p h d", d=65)[:, :, 0:64],
                v_full[:, tt * 1024: tt * 1024 + 1024]
                .rearrange("p (h d) -> p h d", d=64))
            nc.vector.memset(
                v_aug[:, tt * 1040: tt * 1040 + 1040]
                .rearrange("p (h d) -> p h d", d=65)[:, :, 64:65], 1.0)
        mask_sb = attn.tile([128, 1024], bf)
        for ss in range(4):
            nc.sync.dma_start(mask_sb[:, ss * 256: ss * 256 + 256],
                              masks_d[ss])

        proj_cm = tc.tile_pool(name="proj", bufs=1, side="right")
        projp = proj_cm.__enter__()
        catT_sb = projp.tile([128, 4096], bf)    # unnormalized heads^T

        with tc.tile_pool(name="sc", bufs=4, side="left") as scp, \
             tc.tile_pool(name="ps_sc", bufs=2, space="PSUM") as ps_sc, \
             tc.tile_pool(name="ps_av", bufs=3, space="PSUM") as ps_av:
            rc_tiles = []
            # block order within a subtile: non-diag (ph*ss+tl), diag at 2ss+ph
            def blk(ss, ph, tl):
                return 2 * ss + ph if tl == ss else ph * ss + tl
            for j in range(8):
                avps = [ps_av.tile([65, 512], f32, tag="av", name=f"av{j}_{k}")
                        for k in range(2)]
                first = [True, True]
                for ss in range(4):
                    scs = []
                    for h01 in range(2):
                        po = 64 * h01
                        sc_ps = ps_sc.tile([128, 1024], f32, tag="sc",
                                           name=f"sc{j}_{ss}_{h01}")
                        scs.append(sc_ps)
                        for ph in range(2):
                            for tl in range(ss + 1):
                                m = blk(ss, ph, tl)
                                nc.tensor.matmul(
                                    sc_ps[:, m * 128: m * 128 + 128],
                                    kT_full[po:po + 64,
                                            j * 1024 + ph * 512 + tl * 128:
                                            j * 1024 + ph * 512 + tl * 128 + 128],
                                    qT_sb[po:po + 64, j * 512 + ss * 128:
                                          j * 512 + ss * 128 + 128],
                                    start=True, stop=True)
                    exps = []
                    for h01 in range(2):
                        expT = scp.tile([128, 1024], bf, tag="expT",
                                        name=f"ex{j}_{ss}_{h01}")
                        exps.append(expT)
                        nc.scalar.activation(
                            expT[:, 0: (2 * ss + 2) * 128],
                            scs[h01][:, 0: (2 * ss + 2) * 128], AF.Exp)
                        nc.vector.tensor_tensor(
                            expT[:, 2 * ss * 128: 2 * ss * 128 + 256],
                            expT[:, 2 * ss * 128: 2 * ss * 128 + 256],
                            mask_sb[:, ss * 256: ss * 256 + 256], op=ALU.mult)
                    for h01 in range(2):
                        h = 2 * j + h01
                        for ph in range(2):
                            for tl in range(ss + 1):
                                m = blk(ss, ph, tl)
                                u = ph * 4 + tl
                                nc.tensor.matmul(
                                    avps[h01][:, ss * 128: ss * 128 + 128],
                                    v_aug[:, u * 1040 + h * 65:
                                          u * 1040 + h * 65 + 65],
                                    exps[h01][:, m * 128: m * 128 + 128],
                                    start=first[h01],
                                    stop=(ss == 3 and ph == 1 and tl == ss))
                                first[h01] = False
                rcs = []
                for h01 in range(2):
                    nc.scalar.copy(
                        catT_sb[64 * h01:64 * h01 + 64, j * 512: j * 512 + 512],
                        avps[h01][0:64, :])
                    sm_f = scp.tile([1, 512], f32, tag="sm_f", bufs=4,
                                    name=f"smf{j}_{h01}")
                    nc.scalar.copy(sm_f[:], avps[h01][64:65, :])
                    rc_f = scp.tile([1, 512], f32, tag="rc_f", bufs=4,
                                    name=f"rcf{j}_{h01}")
                    nc.vector.reciprocal_approx_fast(rc_f[:], sm_f[:])
                    rc_h = scp.tile([1, 512], bf, tag="rc_h", bufs=4,
                                    name=f"rc{j}_{h01}")
                    nc.vector.tensor_copy(rc_h[:], rc_f[:])
                    rcs.append(rc_h)
                # PE warm fillers (keep HAM at full clock through ACT-bound loop)
                for wi in range(6):
                    wm_ps = ps_av.tile([65, 512], f32, tag="wmv",
                                       name=f"wv{j}_{wi}", bufs=1)
                    nc.tensor.matmul(wm_ps[0:64, :], ident_sb[:, 0:64],
                                     kT_full[:, 0:512], start=True, stop=True)
                # normalize this head-pair's catT block (K=1 bcast matmuls)
                bc_ps = ps_sc.tile([128, 1024], f32, tag="sc",
                                   name=f"bc{j}")[:, 0:512]
                nc.tensor.matmul(bc_ps[0:64, :], ones_row[:, 0:64],
                                 rcs[0][:], start=True, stop=True)
                nc.tensor.matmul(bc_ps[64:128, :], ones_row[:, 0:64],
                                 rcs[1][:], start=True, stop=True)
                nc.vector.tensor_tensor(
                    catT_sb[:, j * 512: j * 512 + 512],
                    catT_sb[:, j * 512: j * 512 + 512], bc_ps[:],
                    op=ALU.mult)
        attn_cm.__exit__(None, None, None)

        # MoE weights prefetch - overlaps proj + LN1 + h-AllGather
        moe_cm = tc.tile_pool(name="moe", bufs=1, side="left")
        moe = moe_cm.__enter__()
        w1_sb = moe.tile([128, 32768], f8)
        w2_sb = moe.tile([128, 32768], f8)
        for et in range(8):
            nc.sync.dma_start(w1_sb[:, et * 4096: et * 4096 + 4096],
                              w1_d[:, et * 4096: et * 4096 + 4096])
        for ft8 in range(8):
            nc.sync.dma_start(w2_sb[:, ft8 * 4096: ft8 * 4096 + 4096],
                              w2_d[:, ft8 * 4096: ft8 * 4096 + 4096])

        wp_sb = projp.tile([128, 8192], bf)
        nc.sync.dma_start(wp_sb[:], wp_d[:])
        x_sb = projp.tile([128, 4096], f32)
        nc.sync.dma_start(x_sb[:], xr_d[:])
        h_sb = projp.tile([128, 4096], bf)
        hT_stage = projp.tile([128, 4096], bf)

        with tc.tile_pool(name="prw", bufs=2, side="left") as prp, \
             tc.tile_pool(name="ps_pr", bufs=4, space="PSUM") as ps_pr:
            for tt in range(4):
                y_sb = prp.tile([128, 1024], f32, tag="y")
                for ec in range(2):
                    ao_ps = ps_pr.tile([128, 512], f32, tag="ao")
                    for jc in range(8):
                        nc.tensor.matmul(
                            ao_ps[:],
                            catT_sb[:, jc * 512 + tt * 128:
                                    jc * 512 + tt * 128 + 128],
                            wp_sb[:, jc * 1024 + ec * 512:
                                  jc * 1024 + ec * 512 + 512],
                            start=(jc == 0), stop=False)
                    nc.tensor.matmul(
                        ao_ps[:], ones_row[:, 0:128],
                        bp_sb[:, ec * 512: ec * 512 + 512],
                        start=False, stop=True)
                    nc.vector.tensor_tensor(
                        y_sb[:, ec * 512: ec * 512 + 512], ao_ps[:],
                        x_sb[:, tt * 1024 + ec * 512: tt * 1024 + ec * 512 + 512],
                        op=ALU.add)
                # LN1 stats
                mean = prp.tile([128, 1], f32, tag="mean")
                nc.vector.reduce_sum(mean[:], y_sb[:], axis=mybir.AxisListType.X)
                nc.vector.tensor_scalar_mul(mean[:], mean[:], 1.0 / 1024.0)
                sq = prp.tile([128, 1024], bf, tag="sq")
                sqs = prp.tile([128, 1], f32, tag="sqs")
                nc.scalar.activation(sq[:], y_sb[:], AF.Square,
                                     accum_out=sqs[:])
                m2 = prp.tile([128, 1], f32, tag="m2")
                nc.scalar.activation(m2[:], mean[:], AF.Square)
                var = prp.tile([128, 1], f32, tag="var")
                nc.vector.tensor_scalar(var[:], sqs[:], 1.0 / 1024.0, EPS,
                                        op0=ALU.mult, op1=ALU.add)
                nc.vector.tensor_tensor(var[:], var[:], m2[:], op=ALU.subtract)
                std = prp.tile([128, 1], f32, tag="std")
                nc.scalar.activation(std[:], var[:], AF.Sqrt)
                rstd = prp.tile([128, 1], f32, tag="rstd")
                nc.vector.reciprocal(rstd[:], std[:])
                nc.vector.tensor_scalar(
                    h_sb[:, tt * 1024: tt * 1024 + 1024], y_sb[:],
                    mean[:], rstd[:], op0=ALU.subtract, op1=ALU.mult)
                # transpose h tile -> hT
                for et in range(8):
                    tp = ps_pr.tile([128, 128], bf, tag="tp")
                    nc.tensor.transpose(
                        tp[:], h_sb[:, tt * 1024 + et * 128:
                                    tt * 1024 + et * 128 + 128], ident_sb[:])
                    nc.scalar.copy(
                        hT_stage[:, et * 512 + tt * 128:
                                 et * 512 + tt * 128 + 128], tp[:])
                if tt == 1:
                    for et in range(8):
                        nc.gpsimd.dma_start(
                            hag_inA[et],
                            hT_stage[:, et * 512: et * 512 + 256])
                    nc.gpsimd.collective_compute(
                        "AllGather", mybir.AluOpType.bypass,
                        replica_groups=[list(range(8))],
                        ins=[hag_inA.opt()], outs=[hag_outA.opt()])
                if tt == 3:
                    for et in range(8):
                        nc.gpsimd.dma_start(
                            hag_inB[et],
                            hT_stage[:, et * 512 + 256: et * 512 + 512])
                    nc.gpsimd.collective_compute(
                        "AllGather", mybir.AluOpType.bypass,
                        replica_groups=[list(range(8))],
                        ins=[hag_inB.opt()], outs=[hag_outB.opt()])
            # (b) PE warm-keepers over the h-AG wait
            for wi in range(16):
                wm_ps = ps_pr.tile([128, 512], f32, tag="ao", name=f"wm{wi}")
                nc.tensor.matmul(wm_ps[:], catT_sb[:, 0:128],
                                 wp_sb[:, 0:512], start=True, stop=True)
        proj_cm.__exit__(None, None, None)

        # ============ phase 3: MoE (expert r over all tokens) ============
        with tc.tile_pool(name="mchunk", bufs=2, side="left") as mck, \
             tc.tile_pool(name="ps_md", bufs=2, space="PSUM") as ps_md, \
             tc.tile_pool(name="ps_eo", bufs=2, space="PSUM") as ps_eo:
            order = []
            for qq in range(4):
                order += [4 * qq, 4 * qq + 2, 4 * qq + 1, 4 * qq + 3]
            gcount = {0: 0, 1: 0, 2: 0, 3: 0}
            for ci in order:
                hT_c = mck.tile([128, 2048], bf, tag="hT_c")
                hsrc = hag_outA if ci % 2 == 0 else hag_outB
                for et in range(8):
                    nc.gpsimd.dma_start(
                        hT_c[:, et * 256: et * 256 + 256],
                        hsrc[ci // 2, et])
                hT8 = mck.tile([128, 2048], f8, tag="hT8")
                nc.vector.tensor_copy(hT8[:], hT_c[:])
                h8r = hT8[:].rearrange("p (et t) -> p et t", et=8)
                pcol = mck.tile([128, 2], f32, tag="pcol")
                for th in range(2):
                    lg_ps = ps_eo.tile([128, 8], f32, tag="lg")
                    for et in range(8):
                        nc.tensor.matmul(
                            lg_ps[:],
                            hT_c[:, et * 256 + th * 128: et * 256 + th * 128 + 128],
                            wr_sb[:, et * 8: et * 8 + 8],
                            start=(et == 0), stop=False)
                    nc.tensor.matmul(lg_ps[:], ones_row[:, 0:128], br_sb[:],
                                     start=False, stop=True)
                    pe = mck.tile([128, 8], f32, tag="pe")
                    ps = mck.tile([128, 1], f32, tag="ps")
                    nc.scalar.activation(pe[:], lg_ps[:], AF.Exp,
                                         accum_out=ps[:])
                    pr = mck.tile([128, 1], f32, tag="pr")
                    nc.vector.reciprocal(pr[:], ps[:])
                    nc.vector.tensor_tensor(pcol[:, th:th + 1], pe[:, 0:1],
                                            pr[:], op=ALU.mult)
                # fold away the x16 fp8 prescale of w1 and w2
                nc.vector.tensor_scalar_mul(pcol[:], pcol[:],
                                            1.0 / (WSCALE * WSCALE))
                midT = mck.tile([128, 8192], f8, tag="midT", bufs=2)
                w1r = w1_sb[:].rearrange("p (et f) -> p et f", et=8)
                for ft in range(32):
                    md_ps = ps_md.tile([128, 256], f32, tag="md")
                    for ep in range(4):
                        nc.tensor.matmul(
                            md_ps[:],
                            w1r[:, 2 * ep: 2 * ep + 2,
                                ft * 128: ft * 128 + 128],
                            h8r[:, 2 * ep: 2 * ep + 2, :],
                            start=(ep == 0), stop=(ep == 3),
                            perf_mode=DR)
                    if ft % 2 == 0:
                        nc.scalar.activation(
                            midT[:, ft * 256: ft * 256 + 256], md_ps[:],
                            AF.Relu, bias=b1_sb[:, ft: ft + 1])
                    else:
                        nc.vector.tensor_scalar(
                            midT[:, ft * 256: ft * 256 + 256], md_ps[:],
                            b1_sb[:, ft: ft + 1], 0.0,
                            op0=ALU.add, op1=ALU.max)
                eo_sb = mck.tile([128, 2048], f32, tag="eo", bufs=2)
                m8r = midT[:].rearrange("p (ft t) -> p ft t", ft=32)
                w2r = w2_sb[:].rearrange("p (ft e) -> p ft e", ft=32)
                for th in range(2):
                    for ec in range(2):
                        eo_ps = ps_eo.tile([128, 512], f32, tag="eo_ps")
                        for fp in range(16):
                            nc.tensor.matmul(
                                eo_ps[:],
                                m8r[:, 2 * fp: 2 * fp + 2,
                                    th * 128: th * 128 + 128],
                                w2r[:, 2 * fp: 2 * fp + 2,
                                    ec * 512: ec * 512 + 512],
                                start=(fp == 0), stop=False,
                                perf_mode=DR)
                        nc.tensor.matmul(
                            eo_ps[:], ones_row[:, 0:128],
                            b2_sb[:, ec * 512: ec * 512 + 512],
                            start=False, stop=True)
                        if ec == 0:
                            nc.scalar.activation(
                                eo_sb[:, th * 1024 + ec * 512:
                                      th * 1024 + ec * 512 + 512],
                                eo_ps[:], AF.Identity,
                                scale=pcol[:, th: th + 1])
                        else:
                            nc.vector.tensor_scalar_mul(
                                eo_sb[:, th * 1024 + ec * 512:
                                      th * 1024 + ec * 512 + 512],
                                eo_ps[:], pcol[:, th: th + 1])
                g, gi = ci // 4, ci % 4
                for th in range(2):
                    nc.sync.dma_start(
                        rs_ins[g][gi * 256 + th * 128: gi * 256 + th * 128 + 128, :],
                        eo_sb[:, th * 1024: th * 1024 + 1024])
                gcount[g] += 1
                if gcount[g] == 4:
                    nc.gpsimd.collective_compute(
                        "ReduceScatter", mybir.AluOpType.add,
                        replica_groups=[list(range(8))],
                        ins=[rs_ins[g].opt()], outs=[rs_outs[g].opt()])
        moe_cm.__exit__(None, None, None)

        # ============ phase 4: residual + LN2 (per RS group/batch) ============
        with tc.tile_pool(name="fin", bufs=2, side="left") as fin:
            x2_sb = fin.tile([128, 4096], f32, bufs=1)
            nc.sync.dma_start(x2_sb[:], xr2_d[:])
            g2_sb = fin.tile([128, 1024], f32, bufs=1)
            nc.sync.dma_start(g2_sb[:], g2_d[:])
            bl2_sb = fin.tile([128, 1024], f32, bufs=1)
            nc.sync.dma_start(bl2_sb[:], bl2_d[:])
            for g in range(4):
                y2 = fin.tile([128, 1024], f32, tag="y2")
                nc.sync.dma_start(y2[:], rs_outs[g][:])
                nc.vector.tensor_tensor(
                    y2[:], y2[:], x2_sb[:, g * 1024: g * 1024 + 1024],
                    op=ALU.add)
                mean = fin.tile([128, 1], f32, tag="mean2")
                nc.vector.reduce_sum(mean[:], y2[:], axis=mybir.AxisListType.X)
                nc.vector.tensor_scalar_mul(mean[:], mean[:], 1.0 / 1024.0)
                sq = fin.tile([128, 1024], f32, tag="sq2")
                sqs = fin.tile([128, 1], f32, tag="sqs2")
                nc.scalar.activation(sq[:], y2[:], AF.Square, accum_out=sqs[:])
                m2 = fin.tile([128, 1], f32, tag="m22")
                nc.scalar.activation(m2[:], mean[:], AF.Square)
                var = fin.tile([128, 1], f32, tag="var2")
                nc.vector.tensor_scalar(var[:], sqs[:], 1.0 / 1024.0, EPS,
                                        op0=ALU.mult, op1=ALU.add)
                nc.vector.tensor_tensor(var[:], var[:], m2[:], op=ALU.subtract)
                std = fin.tile([128, 1], f32, tag="std2")
                nc.scalar.activation(std[:], var[:], AF.Sqrt)
                rstd = fin.tile([128, 1], f32, tag="rstd2")
                nc.vector.reciprocal(rstd[:], std[:])
                on = fin.tile([128, 1024], f32, tag="on")
                nc.vector.tensor_scalar(on[:], y2[:], mean[:], rstd[:],
                                        op0=ALU.subtract, op1=ALU.mult)
                nc.vector.tensor_tensor(on[:], on[:], g2_sb[:], op=ALU.mult)
                nc.vector.tensor_tensor(on[:], on[:], bl2_sb[:], op=ALU.add)
                nc.sync.dma_start(out_d[g], on[:])
        cpool_cm.__exit__(None, None, None)
    nc.compile()
    return nc


def _prep_inputs(inputs):
    f = np.float32
    x = np.asarray(inputs["x"], f)
    wq, bq = np.asarray(inputs["wq"], f), np.asarray(inputs["bq"], f)
    wk, bk = np.asarray(inputs["wk"], f), np.asarray(inputs["bk"], f)
    wv, bv = np.asarray(inputs["wv"], f), np.asarray(inputs["bv"], f)
    wp, bp = np.asarray(inputs["wp"], f), np.asarray(inputs["bp"], f)
    ln1_g, ln1_b = np.asarray(inputs["ln1_g"], f), np.asarray(inputs["ln1_b"], f)
    ln2_g, ln2_b = np.asarray(inputs["ln2_g"], f), np.asarray(inputs["ln2_b"], f)
    wr, br = np.asarray(inputs["wr"], f), np.asarray(inputs["br"], f)
    w1, b1 = np.asarray(inputs["w1"], f), np.asarray(inputs["b1"], f)
    w2, b2 = np.asarray(inputs["w2"], f), np.asarray(inputs["b2"], f)

    def etile(a):  # [E, M] -> [128, 8*M]
        M = a.shape[1]
        return np.ascontiguousarray(
            a.reshape(8, 128, M).transpose(1, 0, 2).reshape(128, 8 * M))

    wq_f = wq.transpose(1, 0, 2).reshape(E, E)   # [e, h*64+d]
    wk_f = wk.transpose(1, 0, 2).reshape(E, E)
    wv_f = wv.transpose(1, 0, 2).reshape(E, E)
    wqkv = np.concatenate([wq_f, wk_f, wv_f], axis=1)        # [E, 3E]
    wqkv_t = etile(wqkv).astype(BF16)                        # [128, 8*3072]
    bqk = np.concatenate([bq.reshape(-1).reshape(8, 128).T,
                          bk.reshape(-1).reshape(8, 128).T], axis=1).astype(f)
    wp_t = etile(wp).astype(BF16)                            # [128, 8*1024]
    w1e = (ln1_g[:, None] * w1).astype(f)                    # [n,E,FF]
    b1e = b1 + ln1_b @ w1                                    # [n,FF]
    wre = (ln1_g[:, None] * wr).astype(f)                    # [E,8]
    bre = br + ln1_b @ wr                                    # [8]
    ident = np.eye(128, dtype=BF16)

    in_maps = []
    for r in range(NCORE):
        b, p = r // 2, r % 2
        # interleaved token assignment: local s_loc <-> orig row 2*s_loc + p
        xs = np.ascontiguousarray(x[b, p::2, :])             # [512, E]
        xpart = np.ascontiguousarray(x[b, 1 - p::2, :])      # partner tokens
        xT_t = np.concatenate(
            [etile(np.ascontiguousarray(xs.T)),
             etile(np.ascontiguousarray(xpart.T))], axis=1).astype(BF16)
        xr_t = np.ascontiguousarray(
            xs.reshape(4, 128, 1024).transpose(1, 0, 2).reshape(128, 4096), f)
        # final-phase x rows: vt rows [128r,128r+128) of every batch
        sv = 128 * r + np.arange(128)
        orig_s = 2 * (sv % 512) + sv // 512
        xr2_t = np.ascontiguousarray(
            x[:, orig_s, :].transpose(1, 0, 2).reshape(128, 4096), f)
        # diagonal causal masks: half0 = own parity keys, half1 = partner
        masks = np.zeros((4, 128, 256), BF16)
        ti = np.arange(128)
        sj = np.arange(128)
        own = (ti[:, None] <= sj[None, :])
        part = (ti[:, None] <= sj[None, :]) if p == 1 else                (ti[:, None] < sj[None, :])
        for ss in range(4):
            masks[ss][:, 0:128] = own.astype(BF16)
            masks[ss][:, 128:256] = part.astype(BF16)
        perm = [r] + [i for i in range(NEXP) if i != r]
        wr_p = etile(wre[:, perm]).astype(BF16)              # [128, 8*8]
        br_p = bre[perm].reshape(1, 8).astype(BF16)
        w1_t = etile(w1e[r] * WSCALE).astype(FP8)            # [128, 8*4096]
        b1_t = np.ascontiguousarray(
            b1e[r].reshape(32, 128).T * WSCALE, f)           # [128, 32]
        w2_t = np.ascontiguousarray(
            w2[r].reshape(32, 128, 1024).transpose(1, 0, 2)
            .reshape(128, 32 * 1024) * WSCALE).astype(FP8)
        in_maps.append({
            "xT": xT_t, "xr": xr_t, "xr2": xr2_t, "wqkv": wqkv_t, "bqk": bqk,
            "bv": bv.reshape(1, E).astype(BF16),
            "wp": wp_t, "bp": bp.reshape(1, E).astype(BF16),
            "masks": masks, "ident": ident,
            "w1": w1_t, "b1": b1_t, "w2": w2_t,
            "b2": (b2[r] * WSCALE * WSCALE).reshape(1, E).astype(BF16),
            "wr": wr_p, "br": br_p,
            "g2": np.broadcast_to(ln2_g, (128, E)).astype(f).copy(),
            "bl2": np.broadcast_to(ln2_b, (128, E)).astype(f).copy(),
        })
    return in_maps


def kernel(**inputs):
    from concourse import bass_utils
    if "nc" not in _CACHE:
        _CACHE["nc"] = _build_program()
    nc = _CACHE["nc"]
    in_maps = _prep_inputs(inputs)
    res = bass_utils.run_bass_kernel_spmd(
        nc, in_maps, core_ids=list(range(NCORE)))
    # core r returns vt rows [128r, 128r+128) of every batch (interleaved map)
    full = np.empty((B, S, E), np.float32)
    for r in range(NCORE):
        o = res.results[r]["out"]                            # [4, 128, 1024]
        sv = 128 * r + np.arange(128)
        orig_s = 2 * (sv % 512) + sv // 512
        full[:, orig_s, :] = o
    return full



# revision 18
# speedup vs baseline: 1.9363x; 1.3025x over previous
"""Trainium2 Bass kernel for nn_Block_56616258896419 (moe_routing).

Self-contained: takes FULL inputs (as from setup_inputs()), returns FULL
[4,1024,1024] f32 output. Internally shards across 8 NeuronCores:
  - tokens 8-way (core r: batch r//2, sequence parity r%2); attention,
    LN1, the full 8-expert MoE (fp8 DoubleRow, weights streamed from
    HBM), and LN2 all run data-parallel on the core's own 512 tokens.
No collectives: partner-token K/V are recomputed locally from x.
"""
import numpy as np
import ml_dtypes

B, S, E, H, HD, NEXP, FF = 4, 1024, 1024, 16, 64, 8, 4096
NCORE = 8
TOK = 512          # tokens per core
TC = 256           # MoE token-chunk
NCHUNK = (B * S) // TC
EPS = 1e-5
BF16 = ml_dtypes.bfloat16
FP8 = ml_dtypes.float8_e4m3
WSCALE = 16.0          # fp8 prescale for w1/w2 (keeps weights in normal range)

_CACHE = {}


def _build_program():
    import concourse.bacc as bacc
    import concourse.mybir as mybir
    import concourse.tile as tile

    dt = mybir.dt
    f32, bf, f8 = dt.float32, dt.bfloat16, dt.float8e4
    AF = mybir.ActivationFunctionType
    ALU = mybir.AluOpType
    DR = mybir.MatmulPerfMode.DoubleRow

    nc = bacc.Bacc("TRN2", target_bir_lowering=False, debug=False,
                   num_devices=NCORE)

    # ---------------- I/O ----------------
    def inp(name, shape, d):
        return nc.dram_tensor(name, shape, d, kind="ExternalInput").ap()

    xT_d = inp("xT", [128, 2 * 4096], bf)         # x^T [own|partner], e-tiled
    xr_d = inp("xr", [128, 4 * 1024], f32)        # x token-major, tt blocks
    wqkv_d = inp("wqkv", [128, 8 * 3072], bf)     # [E,3E] e-tiled
    bqk_d = inp("bqk", [128, 16], f32)
    bv_d = inp("bv", [1, 1024], bf)
    wp_d = inp("wp", [128, 8 * 1024], bf)
    bp_d = inp("bp", [1, 1024], bf)
    masks_d = inp("masks", [4, 128, 256], bf)     # diag-pair 0/1 masks
    ident_d = inp("ident", [128, 128], bf)
    w1_d = inp("w1", [8, 128, 8 * 4096], f8)      # all experts, e-tiled, x16
    b1_d = inp("b1", [128, 8 * 32], f32)          # [ft, n*32+ft] layout, x16
    w2_d = inp("w2", [8, 128, 32 * 1024], f8)     # all experts, ff-tiled, x16
    b2_d = inp("b2", [1, 8 * 1024], bf)           # x256
    wr_d = inp("wr", [128, 8 * 8], f8)            # ln1-folded, x16
    br_d = inp("br", [1, 8], bf)                  # x16
    g2_d = inp("g2", [128, 1024], f32)            # ln2_g replicated
    bl2_d = inp("bl2", [128, 1024], f32)          # ln2_b replicated
    out_d = nc.dram_tensor("out", [4, 128, 1024], f32,
                           kind="ExternalOutput").ap()

    with tile.TileContext(nc) as tc:
        cpool_cm = tc.tile_pool(name="cpool", bufs=1, side="left")
        cpool = cpool_cm.__enter__()
        ones_row = cpool.tile([1, 128], bf)
        nc.vector.memset(ones_row[:], 1.0)
        ones_f = cpool.tile([1, 128], f32)
        nc.vector.memset(ones_f[:], 1.0)
        bqk_sb = cpool.tile([128, 16], f32)
        nc.sync.dma_start(bqk_sb[:], bqk_d[:])
        bv_sb = cpool.tile([1, 1024], bf)
        nc.sync.dma_start(bv_sb[:], bv_d[:])
        bp_sb = cpool.tile([1, 1024], bf)
        nc.sync.dma_start(bp_sb[:], bp_d[:])
        ident_sb = cpool.tile([128, 128], bf)
        nc.sync.dma_start(ident_sb[:], ident_d[:])
        wr_sb = cpool.tile([128, 64], f8)
        nc.sync.dma_start(wr_sb[:], wr_d[:])
        br_sb = cpool.tile([1, 8], bf)
        nc.sync.dma_start(br_sb[:], br_d[:])
        b1_sb = cpool.tile([128, 256], f32)
        nc.sync.dma_start(b1_sb[:], b1_d[:])
        b2_sb = cpool.tile([1, 8192], bf)
        nc.sync.dma_start(b2_sb[:], b2_d[:])

        # ===== phase 1: local K/V for BOTH interleave-halves, then Q =====
        qkv_cm = tc.tile_pool(name="qkv", bufs=1, side="right")
        qkv = qkv_cm.__enter__()
        xT_sb = qkv.tile([128, 8192], bf)
        nc.sync.dma_start(xT_sb[:], xT_d[:])
        wqkv_sb = qkv.tile([128, 24576], bf)
        for sect in (1, 2, 0):            # k first, then v, then q
            nc.sync.dma_start(
                wqkv_sb[:].rearrange("p (e s c) -> p e s c", e=8, s=3)[:, :, sect],
                wqkv_d[:].rearrange("p (e s c) -> p e s c", e=8, s=3)[:, :, sect])

        attn_cm = tc.tile_pool(name="attn", bufs=1, side="left")
        attn = attn_cm.__enter__()
        qT_sb = attn.tile([128, 4096], bf)
        kT_full = attn.tile([128, 8192], bf)     # [j][half*512 + s]
        v_full = attn.tile([128, 8192], bf)      # [u = half*4+tt][hd]

        with tc.tile_pool(name="ps_qkv", bufs=3, space="PSUM") as psq:
            for half in range(2):
                for j in range(8):
                    k_ps = psq.tile([128, 512], f32, tag="qk_ps")
                    for et in range(8):
                        nc.tensor.matmul(
                            k_ps[:],
                            wqkv_sb[:, et * 3072 + 1024 + j * 128:
                                    et * 3072 + 1024 + j * 128 + 128],
                            xT_sb[:, half * 4096 + et * 512:
                                  half * 4096 + et * 512 + 512],
                            start=(et == 0), stop=(et == 7))
                    nc.vector.tensor_scalar(
                        kT_full[:, j * 1024 + half * 512:
                                j * 1024 + half * 512 + 512], k_ps[:],
                        bqk_sb[:, 8 + j: 8 + j + 1], None, op0=ALU.add)
            for half in range(2):
                for tt in range(4):
                    for c in range(2):
                        v_ps = psq.tile([128, 512], f32, tag="v_ps")
                        for et in range(8):
                            nc.tensor.matmul(
                                v_ps[:],
                                xT_sb[:, half * 4096 + et * 512 + tt * 128:
                                      half * 4096 + et * 512 + tt * 128 + 128],
                                wqkv_sb[:, et * 3072 + 2048 + c * 512:
                                        et * 3072 + 2048 + c * 512 + 512],
                                start=(et == 0), stop=False)
                        nc.tensor.matmul(
                            v_ps[:], ones_row[:, 0:128],
                            bv_sb[:, c * 512: c * 512 + 512],
                            start=False, stop=True)
                        u = half * 4 + tt
                        nc.scalar.copy(
                            v_full[:, u * 1024 + c * 512:
                                   u * 1024 + c * 512 + 512], v_ps[:])
            # qT (own tokens = half 0)
            for j in range(8):
                q_ps = psq.tile([128, 512], f32, tag="qk_ps")
                for et in range(8):
                    nc.tensor.matmul(
                        q_ps[:],
                        wqkv_sb[:, et * 3072 + j * 128:
                                et * 3072 + j * 128 + 128],
                        xT_sb[:, et * 512: et * 512 + 512],
                        start=(et == 0), stop=(et == 7))
                nc.vector.tensor_scalar(
                    qT_sb[:, j * 512: j * 512 + 512], q_ps[:],
                    bqk_sb[:, j: j + 1], 0.125, op0=ALU.add, op1=ALU.mult)
        qkv_cm.__exit__(None, None, None)

        # ============ phase 2: attention ============
        # augmented V: per t-tile, 16 heads x (64 v-cols + 1 ones-col)
        v_aug = attn.tile([128, 8 * 1040], bf)
        for tt in range(8):
            nc.vector.tensor_copy(
                v_aug[:, tt * 1040: tt * 1040 + 1040]
                .rearrange("p (h d) -> p h d", d=65)[:, :, 0:64],
                v_full[:, tt * 1024: tt * 1024 + 1024]
                .rearrange("p (h d) -> p h d", d=64))
            nc.vector.memset(
                v_aug[:, tt * 1040: tt * 1040 + 1040]
                .rearrange("p (h d) -> p h d", d=65)[:, :, 64:65], 1.0)
        mask_sb = attn.tile([128, 1024], bf)
        for ss in range(4):
            nc.sync.dma_start(mask_sb[:, ss * 256: ss * 256 + 256],
                              masks_d[ss])

        proj_cm = tc.tile_pool(name="proj", bufs=1, side="right")
        projp = proj_cm.__enter__()
        catT_sb = projp.tile([128, 4096], bf)    # unnormalized heads^T

        with tc.tile_pool(name="sc", bufs=4, side="left") as scp, \
             tc.tile_pool(name="ps_sc", bufs=2, space="PSUM") as ps_sc, \
             tc.tile_pool(name="ps_av", bufs=3, space="PSUM") as ps_av:
            rc_tiles = []
            # block order within a subtile: non-diag (ph*ss+tl), diag at 2ss+ph
            def blk(ss, ph, tl):
                return 2 * ss + ph if tl == ss else ph * ss + tl
            for j in range(8):
                avps = [ps_av.tile([65, 512], f32, tag="av", name=f"av{j}_{k}")
                        for k in range(2)]
                first = [True, True]
                for ss in range(4):
                    scs = []
                    for h01 in range(2):
                        po = 64 * h01
                        sc_ps = ps_sc.tile([128, 1024], f32, tag="sc",
                                           name=f"sc{j}_{ss}_{h01}")
                        scs.append(sc_ps)
                        for ph in range(2):
                            for tl in range(ss + 1):
                                m = blk(ss, ph, tl)
                                nc.tensor.matmul(
                                    sc_ps[:, m * 128: m * 128 + 128],
                                    kT_full[po:po + 64,
                                            j * 1024 + ph * 512 + tl * 128:
                                            j * 1024 + ph * 512 + tl * 128 + 128],
                                    qT_sb[po:po + 64, j * 512 + ss * 128:
                                          j * 512 + ss * 128 + 128],
                                    start=True, stop=True)
                    exps = []
                    for h01 in range(2):
                        expT = scp.tile([128, 1024], bf, tag="expT",
                                        name=f"ex{j}_{ss}_{h01}")
                        exps.append(expT)
                        nc.scalar.activation(
                            expT[:, 0: (2 * ss + 2) * 128],
                            scs[h01][:, 0: (2 * ss + 2) * 128], AF.Exp)
                        nc.vector.tensor_tensor(
                            expT[:, 2 * ss * 128: 2 * ss * 128 + 256],
                            expT[:, 2 * ss * 128: 2 * ss * 128 + 256],
                            mask_sb[:, ss * 256: ss * 256 + 256], op=ALU.mult)
                    for h01 in range(2):
                        h = 2 * j + h01
                        for ph in range(2):
                            for tl in range(ss + 1):
                                m = blk(ss, ph, tl)
                                u = ph * 4 + tl
                                nc.tensor.matmul(
                                    avps[h01][:, ss * 128: ss * 128 + 128],
                                    v_aug[:, u * 1040 + h * 65:
                                          u * 1040 + h * 65 + 65],
                                    exps[h01][:, m * 128: m * 128 + 128],
                                    start=first[h01],
                                    stop=(ss == 3 and ph == 1 and tl == ss))
                                first[h01] = False
                rcs = []
                for h01 in range(2):
                    nc.scalar.copy(
                        catT_sb[64 * h01:64 * h01 + 64, j * 512: j * 512 + 512],
                        avps[h01][0:64, :])
                    sm_f = scp.tile([1, 512], f32, tag="sm_f", bufs=4,
                                    name=f"smf{j}_{h01}")
                    nc.scalar.copy(sm_f[:], avps[h01][64:65, :])
                    rc_f = scp.tile([1, 512], f32, tag="rc_f", bufs=4,
                                    name=f"rcf{j}_{h01}")
                    nc.vector.reciprocal_approx_fast(rc_f[:], sm_f[:])
                    rc_h = scp.tile([1, 512], bf, tag="rc_h", bufs=4,
                                    name=f"rc{j}_{h01}")
                    nc.vector.tensor_copy(rc_h[:], rc_f[:])
                    rcs.append(rc_h)
                # PE warm fillers (keep HAM at full clock through ACT-bound loop)
                for wi in range(6):
                    wm_ps = ps_av.tile([65, 512], f32, tag="wmv",
                                       name=f"wv{j}_{wi}", bufs=1)
                    nc.tensor.matmul(wm_ps[0:64, :], ident_sb[:, 0:64],
                                     kT_full[:, 0:512], start=True, stop=True)
                # normalize this head-pair's catT block (K=1 bcast matmuls)
                bc_ps = ps_sc.tile([128, 1024], f32, tag="sc",
                                   name=f"bc{j}")[:, 0:512]
                nc.tensor.matmul(bc_ps[0:64, :], ones_row[:, 0:64],
                                 rcs[0][:], start=True, stop=True)
                nc.tensor.matmul(bc_ps[64:128, :], ones_row[:, 0:64],
                                 rcs[1][:], start=True, stop=True)
                nc.vector.tensor_tensor(
                    catT_sb[:, j * 512: j * 512 + 512],
                    catT_sb[:, j * 512: j * 512 + 512], bc_ps[:],
                    op=ALU.mult)
        attn_cm.__exit__(None, None, None)

        # MoE state: local h^T (fp8), router probs, expert-sum accumulator
        moe_cm = tc.tile_pool(name="moe", bufs=1, side="left")
        moe = moe_cm.__enter__()
        hT_stage8 = moe.tile([128, 4096], f8)
        eo_acc = moe.tile([128, 4096], f32)
        pcol = moe.tile([128, 32], f32)

        wp_sb = projp.tile([128, 8192], bf)
        nc.sync.dma_start(wp_sb[:], wp_d[:])
        x_sb = projp.tile([128, 4096], f32)
        nc.sync.dma_start(x_sb[:], xr_d[:])
        h_sb = projp.tile([128, 4096], bf)

        with tc.tile_pool(name="prw", bufs=2, side="left") as prp, \
             tc.tile_pool(name="ps_pr", bufs=4, space="PSUM") as ps_pr:
            for tt in range(4):
                y_sb = prp.tile([128, 1024], f32, tag="y")
                for ec in range(2):
                    ao_ps = ps_pr.tile([128, 512], f32, tag="ao")
                    for jc in range(8):
                        nc.tensor.matmul(
                            ao_ps[:],
                            catT_sb[:, jc * 512 + tt * 128:
                                    jc * 512 + tt * 128 + 128],
                            wp_sb[:, jc * 1024 + ec * 512:
                                  jc * 1024 + ec * 512 + 512],
                            start=(jc == 0), stop=False)
                    nc.tensor.matmul(
                        ao_ps[:], ones_row[:, 0:128],
                        bp_sb[:, ec * 512: ec * 512 + 512],
                        start=False, stop=True)
                    nc.vector.tensor_tensor(
                        y_sb[:, ec * 512: ec * 512 + 512], ao_ps[:],
                        x_sb[:, tt * 1024 + ec * 512: tt * 1024 + ec * 512 + 512],
                        op=ALU.add)
                # LN1 stats
                mean = prp.tile([128, 1], f32, tag="mean")
                nc.vector.reduce_sum(mean[:], y_sb[:], axis=mybir.AxisListType.X)
                nc.vector.tensor_scalar_mul(mean[:], mean[:], 1.0 / 1024.0)
                sq = prp.tile([128, 1024], bf, tag="sq")
                sqs = prp.tile([128, 1], f32, tag="sqs")
                nc.scalar.activation(sq[:], y_sb[:], AF.Square,
                                     accum_out=sqs[:])
                m2 = prp.tile([128, 1], f32, tag="m2")
                nc.scalar.activation(m2[:], mean[:], AF.Square)
                var = prp.tile([128, 1], f32, tag="var")
                nc.vector.tensor_scalar(var[:], sqs[:], 1.0 / 1024.0, EPS,
                                        op0=ALU.mult, op1=ALU.add)
                nc.vector.tensor_tensor(var[:], var[:], m2[:], op=ALU.subtract)
                std = prp.tile([128, 1], f32, tag="std")
                nc.scalar.activation(std[:], var[:], AF.Sqrt)
                rstd = prp.tile([128, 1], f32, tag="rstd")
                nc.vector.reciprocal(rstd[:], std[:])
                nc.vector.tensor_scalar(
                    h_sb[:, tt * 1024: tt * 1024 + 1024], y_sb[:],
                    mean[:], rstd[:], op0=ALU.subtract, op1=ALU.mult)
                # transpose h tile -> hT (fp8)
                for et in range(8):
                    tp = ps_pr.tile([128, 128], bf, tag="tp")
                    nc.tensor.transpose(
                        tp[:], h_sb[:, tt * 1024 + et * 128:
                                    tt * 1024 + et * 128 + 128], ident_sb[:])
                    nc.scalar.copy(
                        hT_stage8[:, et * 512 + tt * 128:
                                  et * 512 + tt * 128 + 128], tp[:])
        proj_cm.__exit__(None, None, None)

        # ======= phase 3: MoE — all 8 experts over own 512 tokens =======
        h8r = hT_stage8[:].rearrange("p (et t) -> p et t", et=8)
        with tc.tile_pool(name="wstr", bufs=1, side="right") as wst, \
             tc.tile_pool(name="mchunk", bufs=2, side="left") as mck, \
             tc.tile_pool(name="ps_md", bufs=2, space="PSUM") as ps_md, \
             tc.tile_pool(name="ps_eo", bufs=2, space="PSUM") as ps_eo:
            # router probs for all 4 token-blocks (from fp8 h, wr x16)
            for th in range(4):
                lg_ps = ps_eo.tile([128, 8], f32, tag="lg")
                for et in range(8):
                    nc.tensor.matmul(
                        lg_ps[:],
                        hT_stage8[:, et * 512 + th * 128:
                                  et * 512 + th * 128 + 128],
                        wr_sb[:, et * 8: et * 8 + 8],
                        start=(et == 0), stop=False)
                nc.tensor.matmul(lg_ps[:], ones_row[:, 0:128], br_sb[:],
                                 start=False, stop=True)
                pe = mck.tile([128, 8], f32, tag="pe")
                ps = mck.tile([128, 1], f32, tag="ps")
                nc.scalar.activation(pe[:], lg_ps[:], AF.Exp,
                                     accum_out=ps[:], scale=1.0 / WSCALE)
                pr = mck.tile([128, 1], f32, tag="pr")
                nc.vector.reciprocal(pr[:], ps[:])
                # fold away the x16 fp8 prescale of w1 and w2
                nc.vector.tensor_scalar(
                    pcol[:, th * 8: th * 8 + 8], pe[:], pr[:],
                    1.0 / (WSCALE * WSCALE), op0=ALU.mult, op1=ALU.mult)
            for n in range(NEXP):
                w1_t = wst.tile([128, 32768], f8, tag="w1", bufs=2)
                nc.sync.dma_start(w1_t[:], w1_d[n])
                w1r = w1_t[:].rearrange("p (et f) -> p et f", et=8)
                midT = mck.tile([128, 16384], f8, tag="midT")
                for ft in range(32):
                    md_ps = ps_md.tile([128, 512], f32, tag="md")
                    for ep in range(4):
                        nc.tensor.matmul(
                            md_ps[:],
                            w1r[:, 2 * ep: 2 * ep + 2,
                                ft * 128: ft * 128 + 128],
                            h8r[:, 2 * ep: 2 * ep + 2, :],
                            start=(ep == 0), stop=(ep == 3),
                            perf_mode=DR)
                    if ft % 2 == 0:
                        nc.scalar.activation(
                            midT[:, ft * 512: ft * 512 + 512], md_ps[:],
                            AF.Relu, bias=b1_sb[:, n * 32 + ft: n * 32 + ft + 1])
                    else:
                        nc.vector.tensor_scalar(
                            midT[:, ft * 512: ft * 512 + 512], md_ps[:],
                            b1_sb[:, n * 32 + ft: n * 32 + ft + 1], 0.0,
                            op0=ALU.add, op1=ALU.max)
                w2_t = wst.tile([128, 32768], f8, tag="w2", bufs=1)
                nc.sync.dma_start(w2_t[:], w2_d[n])
                m8r = midT[:].rearrange("p (ft t) -> p ft t", ft=32)
                w2r = w2_t[:].rearrange("p (ft e) -> p ft e", ft=32)
                for th in range(4):
                    for ec in range(2):
                        eo_ps = ps_eo.tile([128, 512], f32, tag="eo_ps")
                        for fp in range(16):
                            nc.tensor.matmul(
                                eo_ps[:],
                                m8r[:, 2 * fp: 2 * fp + 2,
                                    th * 128: th * 128 + 128],
                                w2r[:, 2 * fp: 2 * fp + 2,
                                    ec * 512: ec * 512 + 512],
                                start=(fp == 0), stop=False,
                                perf_mode=DR)
                        nc.tensor.matmul(
                            eo_ps[:], ones_row[:, 0:128],
                            b2_sb[:, n * 1024 + ec * 512:
                                  n * 1024 + ec * 512 + 512],
                            start=False, stop=True)
                        dst = eo_acc[:, th * 1024 + ec * 512:
                                     th * 1024 + ec * 512 + 512]
                        pc = pcol[:, th * 8 + n: th * 8 + n + 1]
                        if n == 0:
                            if ec == 0:
                                nc.scalar.activation(dst, eo_ps[:],
                                                     AF.Identity, scale=pc)
                            else:
                                nc.vector.tensor_scalar_mul(dst, eo_ps[:], pc)
                        else:
                            tmp = mck.tile([128, 512], bf, tag="eotmp",
                                           bufs=2)
                            if ec == 0:
                                nc.scalar.activation(tmp[:], eo_ps[:],
                                                     AF.Identity, scale=pc)
                            else:
                                nc.vector.tensor_scalar_mul(tmp[:], eo_ps[:],
                                                            pc)
                            nc.vector.tensor_tensor(dst, dst, tmp[:],
                                                    op=ALU.add)

        # ============ phase 4: residual + LN2 on own tokens ============
        with tc.tile_pool(name="fin", bufs=2, side="left") as fin:
            g2_sb = fin.tile([128, 1024], f32, bufs=1)
            nc.sync.dma_start(g2_sb[:], g2_d[:])
            bl2_sb = fin.tile([128, 1024], f32, bufs=1)
            nc.sync.dma_start(bl2_sb[:], bl2_d[:])
            x2_sb = fin.tile([128, 4096], f32, bufs=1)
            nc.sync.dma_start(x2_sb[:], xr_d[:])
            for g in range(4):
                y2 = fin.tile([128, 1024], f32, tag="y2")
                nc.vector.tensor_tensor(
                    y2[:], eo_acc[:, g * 1024: g * 1024 + 1024],
                    x2_sb[:, g * 1024: g * 1024 + 1024],
                    op=ALU.add)
                mean = fin.tile([128, 1], f32, tag="mean2")
                nc.vector.reduce_sum(mean[:], y2[:], axis=mybir.AxisListType.X)
                nc.vector.tensor_scalar_mul(mean[:], mean[:], 1.0 / 1024.0)
                sq = fin.tile([128, 1024], f32, tag="sq2")
                sqs = fin.tile([128, 1], f32, tag="sqs2")
                nc.scalar.activation(sq[:], y2[:], AF.Square, accum_out=sqs[:])
                m2 = fin.tile([128, 1], f32, tag="m22")
                nc.scalar.activation(m2[:], mean[:], AF.Square)
                var = fin.tile([128, 1], f32, tag="var2")
                nc.vector.tensor_scalar(var[:], sqs[:], 1.0 / 1024.0, EPS,
                                        op0=ALU.mult, op1=ALU.add)
                nc.vector.tensor_tensor(var[:], var[:], m2[:], op=ALU.subtract)
                std = fin.tile([128, 1], f32, tag="std2")
                nc.scalar.activation(std[:], var[:], AF.Sqrt)
                rstd = fin.tile([128, 1], f32, tag="rstd2")
                nc.vector.reciprocal(rstd[:], std[:])
                on = fin.tile([128, 1024], f32, tag="on")
                nc.vector.tensor_scalar(on[:], y2[:], mean[:], rstd[:],
                                        op0=ALU.subtract, op1=ALU.mult)
                nc.vector.tensor_tensor(on[:], on[:], g2_sb[:], op=ALU.mult)
                nc.vector.tensor_tensor(on[:], on[:], bl2_sb[:], op=ALU.add)
                nc.sync.dma_start(out_d[g], on[:])
        moe_cm.__exit__(None, None, None)
        cpool_cm.__exit__(None, None, None)
    nc.compile()
    return nc


def _prep_inputs(inputs):
    f = np.float32
    x = np.asarray(inputs["x"], f)
    wq, bq = np.asarray(inputs["wq"], f), np.asarray(inputs["bq"], f)
    wk, bk = np.asarray(inputs["wk"], f), np.asarray(inputs["bk"], f)
    wv, bv = np.asarray(inputs["wv"], f), np.asarray(inputs["bv"], f)
    wp, bp = np.asarray(inputs["wp"], f), np.asarray(inputs["bp"], f)
    ln1_g, ln1_b = np.asarray(inputs["ln1_g"], f), np.asarray(inputs["ln1_b"], f)
    ln2_g, ln2_b = np.asarray(inputs["ln2_g"], f), np.asarray(inputs["ln2_b"], f)
    wr, br = np.asarray(inputs["wr"], f), np.asarray(inputs["br"], f)
    w1, b1 = np.asarray(inputs["w1"], f), np.asarray(inputs["b1"], f)
    w2, b2 = np.asarray(inputs["w2"], f), np.asarray(inputs["b2"], f)

    def etile(a):  # [E, M] -> [128, 8*M]
        M = a.shape[1]
        return np.ascontiguousarray(
            a.reshape(8, 128, M).transpose(1, 0, 2).reshape(128, 8 * M))

    wq_f = wq.transpose(1, 0, 2).reshape(E, E)   # [e, h*64+d]
    wk_f = wk.transpose(1, 0, 2).reshape(E, E)
    wv_f = wv.transpose(1, 0, 2).reshape(E, E)
    wqkv = np.concatenate([wq_f, wk_f, wv_f], axis=1)        # [E, 3E]
    wqkv_t = etile(wqkv).astype(BF16)                        # [128, 8*3072]
    bqk = np.concatenate([bq.reshape(-1).reshape(8, 128).T,
                          bk.reshape(-1).reshape(8, 128).T], axis=1).astype(f)
    wp_t = etile(wp).astype(BF16)                            # [128, 8*1024]
    w1e = (ln1_g[:, None] * w1).astype(f)                    # [n,E,FF]
    b1e = b1 + ln1_b @ w1                                    # [n,FF]
    wre = (ln1_g[:, None] * wr).astype(f)                    # [E,8]
    bre = br + ln1_b @ wr                                    # [8]
    ident = np.eye(128, dtype=BF16)

    # shared (identical on every core) MoE weights, fp8 x16
    w1_all = np.stack([etile(w1e[n] * WSCALE) for n in range(NEXP)])
    w1_all = w1_all.astype(FP8)                              # [8,128,8*4096]
    b1_all = np.concatenate(
        [b1e[n].reshape(32, 128).T * WSCALE for n in range(NEXP)],
        axis=1).astype(f)                                    # [128, 8*32]
    w2_all = np.stack(
        [w2[n].reshape(32, 128, 1024).transpose(1, 0, 2)
         .reshape(128, 32 * 1024) * WSCALE for n in range(NEXP)])
    w2_all = w2_all.astype(FP8)                              # [8,128,32*1024]
    b2_all = (b2 * WSCALE * WSCALE).reshape(1, NEXP * E).astype(BF16)
    wr_t = etile(wre * WSCALE).astype(FP8)                   # [128, 8*8]
    br_t = (bre * WSCALE).reshape(1, 8).astype(BF16)
    g2_t = np.broadcast_to(ln2_g, (128, E)).astype(f).copy()
    bl2_t = np.broadcast_to(ln2_b, (128, E)).astype(f).copy()

    in_maps = []
    for r in range(NCORE):
        b, p = r // 2, r % 2
        # interleaved token assignment: local s_loc <-> orig row 2*s_loc + p
        xs = np.ascontiguousarray(x[b, p::2, :])             # [512, E]
        xpart = np.ascontiguousarray(x[b, 1 - p::2, :])      # partner tokens
        xT_t = np.concatenate(
            [etile(np.ascontiguousarray(xs.T)),
             etile(np.ascontiguousarray(xpart.T))], axis=1).astype(BF16)
        xr_t = np.ascontiguousarray(
            xs.reshape(4, 128, 1024).transpose(1, 0, 2).reshape(128, 4096), f)
        # diagonal causal masks: half0 = own parity keys, half1 = partner
        masks = np.zeros((4, 128, 256), BF16)
        ti = np.arange(128)
        sj = np.arange(128)
        own = (ti[:, None] <= sj[None, :])
        part = (ti[:, None] <= sj[None, :]) if p == 1 else                (ti[:, None] < sj[None, :])
        for ss in range(4):
            masks[ss][:, 0:128] = own.astype(BF16)
            masks[ss][:, 128:256] = part.astype(BF16)
        in_maps.append({
            "xT": xT_t, "xr": xr_t, "wqkv": wqkv_t, "bqk": bqk,
            "bv": bv.reshape(1, E).astype(BF16),
            "wp": wp_t, "bp": bp.reshape(1, E).astype(BF16),
            "masks": masks, "ident": ident,
            "w1": w1_all, "b1": b1_all, "w2": w2_all, "b2": b2_all,
            "wr": wr_t, "br": br_t,
            "g2": g2_t, "bl2": bl2_t,
        })
    return in_maps


def kernel(**inputs):
    from concourse import bass_utils
    if "nc" not in _CACHE:
        _CACHE["nc"] = _build_program()
    nc = _CACHE["nc"]
    in_maps = _prep_inputs(inputs)
    res = bass_utils.run_bass_kernel_spmd(
        nc, in_maps, core_ids=list(range(NCORE)))
    # core r returns its own 512 tokens: batch r//2, parity r%2
    full = np.empty((B, S, E), np.float32)
    for r in range(NCORE):
        o = res.results[r]["out"]                            # [4, 128, 1024]
        full[r // 2, (r % 2)::2, :] = o.reshape(512, E)
    return full

